# revision 15
# baseline (speedup 1.0000x reference)
"""Trainium2 Bass kernel for nn_AttentionV1 (spatial attention block).

Reference computation (per batch b):
    q = wq @ x + bq            [128, 4096]
    k = wk @ aux + bk          [128, 4096]
    v = wv @ x + bv            [128, 4096]
    s = k^T q                  [4096 k, 4096 q]
    a = softmax(s, axis=q)     (normalize across QUERIES for each key row)
    y = v @ a                  [128, 4096]
    z = wz @ y + bz + x        [256, 4096]

Sharding: 8 cores = 4 batches x 2 query-halves.  Each core owns 2048 query
columns of one batch and computes K / V^T for all 4096 keys.  The softmax
normalization axis (q) is sharded: per 512-key chunk the two cores of a
pair AllReduce their exp-rowsums (a 2 KB message).

Pipeline design (v2):
  - S scores in f32r (precision: logits have std ~11, softmax acts like an
    argmax, so Q/K/S must stay near-fp32).  E = exp(S), V^T and the V path
    are bf16 (post-softmax values tolerate 0.4% rounding).
  - One [128, 2048] PSUM slot per 128-key subchunk -> ONE 2048-wide exp
    ACTIVATE (+ one accumulator read) instead of two 1024-wide ones.
  - PSUM = a single rotating pool of two [128, 2048] slots (all 8 banks).
    S subchunks, the per-chunk y accumulators, the late-K projection and
    the V projection all allocate from the same rotation.
  - V^T is produced via DMA-xbar transposes (SBUF->SBUF, bf16) queued in
    the head phase -- no PE transposes, no PSUM pressure in steady state.
  - Per-chunk rowsum AllReduce with ~2 chunks of latency slack before the
    y matmuls for that chunk fire.
  - DMA order is arranged so the first exp can start ~11us in; the z
    projection + residual drains per query-tile right behind the last y.
"""

import sys

if "/opt/trn_rl_repo" not in sys.path:
    sys.path.insert(0, "/opt/trn_rl_repo")

import numpy as np

import concourse.bass as bass  # noqa: F401  (import keeps bass registered)
import concourse.mybir as mybir
import concourse.tile as tile
from concourse import bacc
from concourse import bass2jax
from concourse.masks import make_identity

F32 = mybir.dt.float32
F32R = mybir.dt.float32r
BF16 = mybir.dt.bfloat16
AF = mybir.ActivationFunctionType
ALU = mybir.AluOpType

# Problem constants (hardcoded per harness contract).
B, C = 4, 256
CH = 128          # C // 2, the qkv channel count == SBUF partition count
N = 4096          # H * W
NQ = 2048         # query columns per core (N / 2)
NCORES = 8
NCHUNK = 8        # key chunks of 512
NSUB = 4          # 128-row subchunks per key chunk
QT = 512          # matmul moving-dim tile
EXP_BIAS = -40.0  # constant shift inside exp() to avoid fp32 overflow

GROUPS = [[0, 1], [2, 3], [4, 5], [6, 7]]


def build_program():
    nc = bacc.Bacc("TRN2", target_bir_lowering=False, debug=False,
                   num_devices=NCORES)

    xq_d = nc.dram_tensor("xq", [C, NQ], F32R, kind="ExternalInput")
    aux_d = nc.dram_tensor("aux", [C, N], F32R, kind="ExternalInput")
    xb_d = nc.dram_tensor("xb", [C, N], BF16, kind="ExternalInput")
    wqT_d = nc.dram_tensor("wqT", [C, CH], F32R, kind="ExternalInput")
    wkT_d = nc.dram_tensor("wkT", [C, CH], F32R, kind="ExternalInput")
    wvT_d = nc.dram_tensor("wvT", [C, CH], BF16, kind="ExternalInput")
    wzT_d = nc.dram_tensor("wzT", [CH, C], F32R, kind="ExternalInput")
    bq_d = nc.dram_tensor("bq", [CH, 1], F32, kind="ExternalInput")
    bk_d = nc.dram_tensor("bk", [CH, 1], F32, kind="ExternalInput")
    bv_d = nc.dram_tensor("bv", [CH, 1], F32, kind="ExternalInput")
    bz_d = nc.dram_tensor("bz", [C, 1], F32, kind="ExternalInput")
    z_d = nc.dram_tensor("z", [C, NQ], F32, kind="ExternalOutput")

    with tile.TileContext(nc) as tc:
        with (
            tc.tile_pool(name="const", bufs=1) as constp,
            tc.tile_pool(name="persist", bufs=1) as persist,
            tc.tile_pool(name="dram", bufs=10, space="DRAM") as dramp,
            tc.tile_pool(name="E", bufs=21) as Ep,
            tc.tile_pool(name="rp", bufs=4) as rp,
            tc.tile_pool(name="auxp", bufs=5) as auxp,
            tc.tile_pool(name="xbp", bufs=5) as xbp,
            tc.tile_pool(name="vch", bufs=4) as vchp,
            tc.tile_pool(name="zt", bufs=4) as ztp,
        ):
            # ---- exp table primer: load the ACT exp tables at t=0 so the
            # ~2.7us table load overlaps the input DMAs.
            ebias = constp.tile([128, 1], F32, tag="ebias", name="ebias")
            nc.vector.memset(ebias[:], EXP_BIAS)
            prim = constp.tile([128, 1], F32, tag="prim", name="prim")
            nc.vector.memset(prim[:], 0.0)
            primo = constp.tile([128, 1], F32, tag="primo", name="primo")
            nc.scalar.activation(primo[:], prim[:], AF.Exp, bias=ebias[:],
                                 scale=1.0)

            # ---- collective warm-up: absorbs first-collective setup +
            # core-start skew while the input DMAs proceed.
            warm_sb = constp.tile([1, 4], F32, tag="warm", name="warm_sb")
            nc.vector.memset(warm_sb[:], 1.0)
            warm_in = dramp.tile([1, 4], F32, tag="warmin", name="warmin")
            warm_out = dramp.tile([1, 4], F32, tag="warmout", name="warmout")
            nc.sync.dma_start(warm_in[:], warm_sb[:])
            nc.gpsimd.collective_compute(
                "AllReduce", ALU.add, replica_groups=GROUPS,
                ins=[warm_in.opt()], outs=[warm_out.opt()])

            # ---- constant tiles ----
            wqT = [constp.tile([128, CH], F32R, tag=f"wq{i}", name=f"wq{i}")
                   for i in range(2)]
            wkT = [constp.tile([128, CH], F32R, tag=f"wk{i}", name=f"wk{i}")
                   for i in range(2)]
            wvT = [constp.tile([128, CH], BF16, tag=f"wv{i}", name=f"wv{i}")
                   for i in range(2)]
            wzT_sb = constp.tile([128, C], F32R, tag="wz", name="wzT_sb")
            bq_sb = constp.tile([CH, 1], F32, tag="bq", name="bq_sb")
            bk_sb = constp.tile([CH, 1], F32, tag="bk", name="bk_sb")
            bv_sb = constp.tile([CH, 1], F32, tag="bv", name="bv_sb")
            bz_sb = [constp.tile([128, 1], F32, tag=f"bz{i}", name=f"bz{i}")
                     for i in range(2)]
            ident0 = constp.tile([128, 128], F32, tag="ident0", name="ident0")
            make_identity(nc, ident0[:])
            ident = constp.tile([128, 128], BF16, tag="ident", name="ident")
            nc.vector.tensor_copy(ident[:], ident0[:])

            # ---- persistent activations ----
            xq_sb = [persist.tile([128, NQ], F32R, tag=f"xq{i}",
                                  name=f"xq{i}") for i in range(2)]
            K_sb = persist.tile([128, N], F32R, tag="K", name="K_sb")
            Q_sb = persist.tile([128, NQ], F32R, tag="Q", name="Q_sb")
            Vt = [persist.tile([128, CH], BF16, tag=f"vt{g}", name=f"vt{g}")
                  for g in range(32)]
            y_sb = persist.tile([128, NQ], F32R, tag="y", name="y_sb")

            # ---- input DMAs, priority-ordered (sync queue) ----
            # Critical path to the first exp: wk, wq, aux cols 0:512, xq.
            for i in range(2):
                nc.sync.dma_start(wkT[i][:], wkT_d[i * 128:(i + 1) * 128, :])
            for i in range(2):
                nc.sync.dma_start(wqT[i][:], wqT_d[i * 128:(i + 1) * 128, :])
            nc.sync.dma_start(bk_sb[:], bk_d[:, :])
            nc.sync.dma_start(bq_sb[:], bq_d[:, :])
            aux_t = {}  # (tile_idx, ci) -> sbuf piece
            def load_aux(t):
                for i in range(2):
                    a = auxp.tile([128, QT], F32R, tag=f"a{i}",
                                  name=f"aux{t}_{i}")
                    nc.sync.dma_start(
                        a[:], aux_d[i * 128:(i + 1) * 128,
                                    t * QT:(t + 1) * QT])
                    aux_t[(t, i)] = a
            load_aux(0)
            # xq in 512-column pieces so each Q projection tile can start
            # as soon as its slice lands.
            for t in range(4):
                for i in range(2):
                    nc.sync.dma_start(
                        xq_sb[i][:, t * QT:(t + 1) * QT],
                        xq_d[i * 128:(i + 1) * 128, t * QT:(t + 1) * QT])
            # Secondary: remaining aux tiles, V-path inputs, z weights.
            for t in (1, 2, 3):
                load_aux(t)
            nc.sync.dma_start(bv_sb[:], bv_d[:, :])
            for i in range(2):
                nc.sync.dma_start(wvT[i][:], wvT_d[i * 128:(i + 1) * 128, :])
            xb_t = {}
            def load_xb(kc):
                for i in range(2):
                    xbt = xbp.tile([128, QT], BF16, tag=f"xb{i}",
                                   name=f"xb{kc}_{i}")
                    nc.sync.dma_start(
                        xbt[:], xb_d[i * 128:(i + 1) * 128,
                                     kc * QT:(kc + 1) * QT])
                    xb_t[(kc, i)] = xbt
            for kc in range(4):
                load_xb(kc)
            for t in (4, 5, 6, 7):
                load_aux(t)
            for kc in range(4, 8):
                load_xb(kc)
            nc.sync.dma_start(wzT_sb[:], wzT_d[:, :])
            for i in range(2):
                nc.sync.dma_start(bz_sb[i][:], bz_d[i * 128:(i + 1) * 128, :])

            # ---- head projections: K tile 0 + all of Q (own PSUM pool,
            # closed before the main slot pool claims all 8 banks).
            def proj_tile(ps, w01, src01, bias, dst, dsl):
                nc.tensor.matmul(ps[:], w01[0][:], src01[0][:],
                                 start=True, stop=False)
                nc.tensor.matmul(ps[:], w01[1][:], src01[1][:],
                                 start=False, stop=True)
                nc.vector.tensor_scalar_add(dst[:, dsl], ps[:], bias[:])

            with tc.tile_pool(name="hd_ps", bufs=2, space="PSUM") as hdps:
                ps = hdps.tile([128, QT], F32, tag="hd", name="hd_k0")
                proj_tile(ps, wkT, [aux_t[(0, 0)], aux_t[(0, 1)]], bk_sb,
                          K_sb, slice(0, QT))
                for t in range(4):
                    ps = hdps.tile([128, QT], F32, tag="hd", name=f"hd_q{t}")
                    sl = slice(t * QT, (t + 1) * QT)
                    proj_tile(ps, wqT,
                              [xq_sb[0][:, sl], xq_sb[1][:, sl]], bq_sb,
                              Q_sb, sl)

            # ---- main pipeline ----
            # One PSUM pool: two [128, 2048] slots (all 8 banks).  The
            # rotation carries S subchunks, V / late-K projections, the
            # per-chunk y accumulators, and finally the z projections.
            # Every slot's consumers are emitted immediately after its
            # allocation so the 2-buf rotation can never deadlock.
            slotp_cm = tc.tile_pool(name="slot", bufs=2, space="PSUM")
            slotp = slotp_cm.__enter__()

            def slot_alloc(name):
                return slotp.tile([128, 4 * QT], F32, tag="slot", name=name)

            # V projection block: one slot covers 2 key chunks -- for each,
            # a V matmul pair into one quarter, then four PE transposes of
            # the biased bf16 V into the adjacent quarter, evacuated to the
            # persistent Vt tiles.
            def emit_v_block(kcs, name):
                vslot = slot_alloc(name)
                for j, kc in enumerate(kcs):
                    qsl = slice(2 * j * QT, (2 * j + 1) * QT)
                    nc.tensor.matmul(vslot[:, qsl], wvT[0][:],
                                     xb_t[(kc, 0)][:], start=True, stop=False)
                    nc.tensor.matmul(vslot[:, qsl], wvT[1][:],
                                     xb_t[(kc, 1)][:], start=False, stop=True)
                for j, kc in enumerate(kcs):
                    qsl = slice(2 * j * QT, (2 * j + 1) * QT)
                    vch = vchp.tile([128, QT], BF16, tag="vch",
                                    name=f"vch{kc}")
                    nc.vector.tensor_scalar_add(vch[:], vslot[:, qsl],
                                                bv_sb[:])
                    tq = vslot[:, (2 * j + 1) * QT:(2 * j + 2) * QT]
                    tq16 = tq.bitcast(BF16)
                    for s in range(NSUB):
                        nc.tensor.transpose(tq16[:, s * 128:(s + 1) * 128],
                                            vch[:, s * 128:(s + 1) * 128],
                                            ident[:])
                    for s in range(NSUB):
                        g = kc * NSUB + s
                        nc.vector.tensor_copy(
                            Vt[g][:], tq16[:, s * 128:(s + 1) * 128])

            # Late-K projection block: one slot covers up to 4 K tiles.
            def emit_kl_block(ts, name):
                kslot = slot_alloc(name)
                for j, t in enumerate(ts):
                    qsl = slice(j * QT, (j + 1) * QT)
                    nc.tensor.matmul(kslot[:, qsl], wkT[0][:],
                                     aux_t[(t, 0)][:], start=True, stop=False)
                    nc.tensor.matmul(kslot[:, qsl], wkT[1][:],
                                     aux_t[(t, 1)][:], start=False, stop=True)
                for j, t in enumerate(ts):
                    qsl = slice(j * QT, (j + 1) * QT)
                    nc.vector.tensor_scalar_add(
                        K_sb[:, t * QT:(t + 1) * QT], kslot[:, qsl], bk_sb[:])

            E_c = {}        # chunk -> list of 4 E strips
            r_c = {}        # chunk -> r partial tile [128, 4]
            ar_out = {}     # AR name -> (rout dram tile, kcs)
            rred_c = {}     # chunk -> (reduced rowsum tile, col offset)
            y_c = {}        # chunk -> open y slot

            # Rowsum AllReduce, split into a trigger (staging DMAs on the
            # idle sync queue + gpsimd collective kick) and a deferred
            # readback, so collectives overlap in flight instead of
            # serializing the gpsimd queue on each one's completion.
            def ar_trigger(kcs, name):
                n = len(kcs)
                rin = dramp.tile([128, NSUB * n], F32, tag=f"rin{name}",
                                 name=f"rin{name}")
                rout = dramp.tile([128, NSUB * n], F32, tag=f"rout{name}",
                                  name=f"rout{name}")
                for j, c in enumerate(kcs):
                    nc.sync.dma_start(rin[:, NSUB * j:NSUB * (j + 1)],
                                      r_c[c][:])
                nc.gpsimd.collective_compute(
                    "AllReduce", ALU.add, replica_groups=GROUPS,
                    ins=[rin.opt()], outs=[rout.opt()])
                ar_out[name] = (rout, kcs)

            def ar_read(name):
                rout, kcs = ar_out.pop(name)
                n = len(kcs)
                rr = rp.tile([128, NSUB * n], F32, tag=f"rred{name}",
                             name=f"rred{name}")
                nc.gpsimd.dma_start(rr[:], rout[:])
                for j, c in enumerate(kcs):
                    rred_c[c] = (rr, NSUB * j)

            def emit_scale(kc):
                rr, off = rred_c.pop(kc)
                rinv = rp.tile([128, NSUB], F32, tag="rinv",
                               name=f"rinv{kc}")
                nc.vector.reciprocal(rinv[:], rr[:, off:off + NSUB])
                for s in range(NSUB):
                    g = kc * NSUB + s
                    nc.vector.tensor_scalar_mul(Vt[g][:], Vt[g][:],
                                                rinv[:, s:s + 1])

            def emit_y_qt(kc, qt):
                yslot = y_c[kc]
                qsl = slice(qt * QT, (qt + 1) * QT)
                E = E_c[kc]
                for s in range(NSUB):
                    nc.tensor.matmul(yslot[:, qsl], Vt[kc * NSUB + s][:],
                                     E[s][:, qsl],
                                     start=(s == 0), stop=(s == 3))
                if kc == 0:
                    nc.vector.tensor_copy(y_sb[:, qsl], yslot[:, qsl])
                else:
                    nc.vector.tensor_add(y_sb[:, qsl],
                                         y_sb[:, qsl].bitcast(F32),
                                         yslot[:, qsl])

            def emit_y_first(kc):
                y_c[kc] = slot_alloc(f"y{kc}")
                emit_y_qt(kc, 0)
                emit_y_qt(kc, 1)

            def emit_y_second(kc):
                emit_y_qt(kc, 2)
                emit_y_qt(kc, 3)
                del y_c[kc]
                del E_c[kc]

            # ---- attention loop ----
            for kc in range(NCHUNK):
                if kc >= 5:
                    emit_y_second(kc - 5)
                E = [Ep.tile([128, NQ], BF16, tag="E", name=f"E{kc}_{s}")
                     for s in range(NSUB)]
                E_c[kc] = E
                rc = rp.tile([128, NSUB], F32, tag="rpart", name=f"rc{kc}")
                r_c[kc] = rc
                for s in range(NSUB):
                    ksl = slice((kc * NSUB + s) * 128,
                                (kc * NSUB + s + 1) * 128)
                    sslot = slot_alloc(f"s{kc}_{s}")
                    for j in range(4):
                        qsl = slice(j * QT, (j + 1) * QT)
                        nc.tensor.matmul(sslot[:, qsl], K_sb[:, ksl],
                                         Q_sb[:, qsl], start=True, stop=True)
                    nc.scalar.activation(E[s][:], sslot[:], AF.Exp,
                                         bias=ebias[:], scale=1.0,
                                         accum_out=rc[:, s:s + 1])
                    if kc >= 4 and s == 2:
                        emit_scale(kc - 4)
                    if kc == 0 and s == 1:
                        emit_v_block([0, 1], "vA")
                    if kc == 0 and s == 3:
                        emit_kl_block([1, 2, 3], "klA")
                    if kc == 1 and s == 1:
                        emit_v_block([2, 3], "vB")
                    if kc == 1 and s == 3:
                        emit_kl_block([4, 5], "klB")
                    if kc == 2 and s == 2:
                        emit_v_block([4, 5], "vC")
                    if kc == 3 and s == 2:
                        emit_kl_block([6, 7], "klC")
                    if kc == 3 and s == 3:
                        emit_v_block([6, 7], "vD")
                # chunk boundary: collectives and lagged y sessions
                if kc == 1:
                    ar_trigger([0, 1], "A")
                if kc == 3:
                    ar_trigger([2, 3], "B")
                    ar_read("A")
                if kc == 5:
                    ar_read("B")
                if kc == 6:
                    ar_trigger([4, 5, 6], "C")
                if kc == 7:
                    ar_trigger([7], "D")
                    ar_read("C")
                if kc >= 4:
                    emit_y_first(kc - 4)

            # ---- tail ----
            emit_y_second(3)
            emit_scale(4)
            emit_y_first(4)
            emit_y_second(4)
            emit_scale(5)
            emit_y_first(5)
            emit_y_second(5)
            emit_scale(6)
            emit_y_first(6)
            emit_y_second(6)
            ar_read("D")
            emit_scale(7)
            emit_y_first(7)
            emit_y_second(7)

            # z projection + residual, per 512-query tile
            for half in range(2):
                zslot = slot_alloc(f"z{half}")
                for qi in range(2):
                    qt = half * 2 + qi
                    qsl = slice(qt * QT, (qt + 1) * QT)
                    for co in range(2):
                        psl = slice((qi * 2 + co) * QT,
                                    (qi * 2 + co + 1) * QT)
                        nc.tensor.matmul(
                            zslot[:, psl],
                            wzT_sb[:, co * 128:(co + 1) * 128],
                            y_sb[:, qsl], start=True, stop=True)
                        zt = ztp.tile([128, QT], F32, tag="zt",
                                      name=f"zt{qt}_{co}")
                        nc.vector.scalar_tensor_tensor(
                            zt[:], zslot[:, psl], bz_sb[co][:],
                            xq_sb[co][:, qsl].bitcast(F32),
                            op0=ALU.add, op1=ALU.add)
                        nc.sync.dma_start(
                            z_d[co * 128:(co + 1) * 128, qsl], zt[:])

            slotp_cm.__exit__(None, None, None)

    nc.compile()
    return nc


def make_in_maps(inputs: dict) -> list:
    import ml_dtypes
    x = np.ascontiguousarray(np.asarray(inputs["x"], np.float32)
                             .reshape(B, C, N))
    aux = np.ascontiguousarray(np.asarray(inputs["aux"], np.float32)
                               .reshape(B, C, N))
    xb = x.astype(ml_dtypes.bfloat16)
    wqT = np.ascontiguousarray(np.asarray(inputs["wq_w"], np.float32).T)
    wkT = np.ascontiguousarray(np.asarray(inputs["wk_w"], np.float32).T)
    wvT = np.ascontiguousarray(
        np.asarray(inputs["wv_w"], np.float32).T.astype(ml_dtypes.bfloat16))
    wzT = np.ascontiguousarray(np.asarray(inputs["wz_w"], np.float32).T)
    bq = np.asarray(inputs["wq_b"], np.float32).reshape(CH, 1)
    bk = np.asarray(inputs["wk_b"], np.float32).reshape(CH, 1)
    bv = np.asarray(inputs["wv_b"], np.float32).reshape(CH, 1)
    bz = np.asarray(inputs["wz_b"], np.float32).reshape(C, 1)
    in_maps = []
    for c in range(NCORES):
        b, h = c // 2, c % 2
        in_maps.append({
            "xq": np.ascontiguousarray(x[b][:, h * NQ:(h + 1) * NQ]),
            "aux": aux[b],
            "xb": xb[b],
            "wqT": wqT, "wkT": wkT, "wvT": wvT, "wzT": wzT,
            "bq": bq, "bk": bk, "bv": bv, "bz": bz,
        })
    return in_maps


class Runner:
    """Compile once, then run the SPMD kernel any number of times.

    Mirrors bass2jax.run_bass_via_pjrt's multi-core branch but keeps the
    jitted executable so repeated calls don't re-trace/re-compile.
    """

    def __init__(self, nc=None):
        import jax
        from jax.experimental.shard_map import shard_map
        from jax.sharding import Mesh, PartitionSpec

        self.nc = nc if nc is not None else build_program()
        bass2jax.install_neuronx_cc_hook()
        nc = self.nc
        assert nc.dbg_addr is None
        partition_name = (nc.partition_id_tensor.name
                          if nc.partition_id_tensor else None)

        in_names, out_names, out_avals, zero_outs = [], [], [], []
        for alloc in nc.m.functions[0].allocations:
            if not isinstance(alloc, mybir.MemoryLocationSet):
                continue
            name = alloc.memorylocations[0].name
            if alloc.kind == "ExternalInput":
                if name != partition_name:
                    in_names.append(name)
            elif alloc.kind == "ExternalOutput":
                out_names.append(name)
                shape = tuple(alloc.tensor_shape)
                dtype = mybir.dt.np(alloc.dtype)
                out_avals.append(jax.core.ShapedArray(shape, dtype))
                zero_outs.append(np.zeros(shape, dtype))
        self.in_names = list(in_names)
        self.out_names = out_names
        self.out_avals = out_avals
        n_params = len(in_names)
        n_outs = len(out_avals)
        all_names = in_names + out_names
        if partition_name is not None:
            all_names = all_names + [partition_name]

        def _body(*args):
            operands = list(args)
            if partition_name is not None:
                operands.append(bass2jax.partition_id_tensor())
            outs = bass2jax._bass_exec_p.bind(
                *operands,
                out_avals=tuple(out_avals),
                in_names=tuple(all_names),
                out_names=tuple(out_names),
                lowering_input_output_aliases=(),
                sim_require_finite=True,
                sim_require_nnan=True,
                nc=nc,
            )
            return tuple(outs)

        devices = jax.devices()[:NCORES]
        mesh = Mesh(np.asarray(devices), ("core",))
        from jax.sharding import NamedSharding
        self._sharding = NamedSharding(mesh, PartitionSpec("core"))
        in_specs = (PartitionSpec("core"),) * (n_params + n_outs)
        out_specs = (PartitionSpec("core"),) * n_outs
        self._sharded = jax.jit(
            shard_map(_body, mesh=mesh, in_specs=in_specs,
                      out_specs=out_specs, check_rep=False),
            donate_argnums=tuple(range(n_params, n_params + n_outs)),
            keep_unused=True,
        )
        self._zero_outs = zero_outs

    def device_inputs(self, in_maps):
        """Transfer the concatenated per-core inputs to the devices once."""
        import jax

        concat_in = [
            np.concatenate([np.asarray(in_maps[c][name])
                            for c in range(NCORES)], axis=0)
            for name in self.in_names
        ]
        return [jax.device_put(a, self._sharding) for a in concat_in]

    def run_device(self, dev_in):
        """Execute with device-resident inputs; returns device arrays."""
        concat_zeros = [
            np.zeros((NCORES * z.shape[0], *z.shape[1:]), z.dtype)
            for z in self._zero_outs
        ]
        return self._sharded(*dev_in, *concat_zeros)

    def run(self, in_maps):
        out_arrs = self.run_device(self.device_inputs(in_maps))
        return [
            {
                name: np.asarray(out_arrs[i]).reshape(
                    NCORES, *self.out_avals[i].shape)[c]
                for i, name in enumerate(self.out_names)
            }
            for c in range(NCORES)
        ]


_RUNNER = None


def get_runner() -> Runner:
    global _RUNNER
    if _RUNNER is None:
        _RUNNER = Runner()
    return _RUNNER


def assemble(results) -> np.ndarray:
    out = np.empty((B, C, N), np.float32)
    for c in range(NCORES):
        b, h = c // 2, c % 2
        out[b][:, h * NQ:(h + 1) * NQ] = results[c]["z"]
    return out.reshape(B, C, 64, 64)


def kernel(**inputs) -> np.ndarray:
    runner = get_runner()
    results = runner.run(make_in_maps(inputs))
    return assemble(results)


# revision 18
# speedup vs baseline: 1.1690x; 1.1690x over previous
"""Trainium2 Bass kernel for nn_AttentionV1 (spatial attention block).

Reference computation (per batch b):
    q = wq @ x + bq            [128, 4096]
    k = wk @ aux + bk          [128, 4096]
    v = wv @ x + bv            [128, 4096]
    s = k^T q                  [4096 k, 4096 q]
    a = softmax(s, axis=q)     (normalize across QUERIES for each key row)
    y = v @ a                  [128, 4096]
    z = wz @ y + bz + x        [256, 4096]

Sharding: 8 cores = 4 batches x 2 query-halves.  Each core owns 2048 query
columns of one batch and computes K / V^T for all 4096 keys.  The softmax
normalization axis (q) is sharded: per 512-key chunk the two cores of a
pair AllReduce their exp-rowsums (a 2 KB message).

Pipeline design (v2):
  - S scores in f32r (precision: logits have std ~11, softmax acts like an
    argmax, so Q/K/S must stay near-fp32).  E = exp(S), V^T and the V path
    are bf16 (post-softmax values tolerate 0.4% rounding).
  - One [128, 2048] PSUM slot per 128-key subchunk -> ONE 2048-wide exp
    ACTIVATE (+ one accumulator read) instead of two 1024-wide ones.
  - PSUM = a single rotating pool of two [128, 2048] slots (all 8 banks).
    S subchunks, the per-chunk y accumulators, the late-K projection and
    the V projection all allocate from the same rotation.
  - V^T is produced via DMA-xbar transposes (SBUF->SBUF, bf16) queued in
    the head phase -- no PE transposes, no PSUM pressure in steady state.
  - Per-chunk rowsum AllReduce with ~2 chunks of latency slack before the
    y matmuls for that chunk fire.
  - DMA order is arranged so the first exp can start ~11us in; the z
    projection + residual drains per query-tile right behind the last y.
"""

import sys

if "/opt/trn_rl_repo" not in sys.path:
    sys.path.insert(0, "/opt/trn_rl_repo")

import numpy as np

import concourse.bass as bass  # noqa: F401  (import keeps bass registered)
import concourse.mybir as mybir
import concourse.tile as tile
from concourse import bacc
from concourse import bass2jax
from concourse.masks import make_identity

F32 = mybir.dt.float32
F32R = mybir.dt.float32r
BF16 = mybir.dt.bfloat16
AF = mybir.ActivationFunctionType
ALU = mybir.AluOpType

# Problem constants (hardcoded per harness contract).
B, C = 4, 256
CH = 128          # C // 2, the qkv channel count == SBUF partition count
N = 4096          # H * W
NQ = 2048         # query columns per core (N / 2)
NCORES = 8
NCHUNK = 8        # key chunks of 512
NSUB = 4          # 128-row subchunks per key chunk
QT = 512          # matmul moving-dim tile
EXP_BIAS = -40.0  # constant shift inside exp() to avoid fp32 overflow

GROUPS = [[0, 1], [2, 3], [4, 5], [6, 7]]


def build_program():
    nc = bacc.Bacc("TRN2", target_bir_lowering=False, debug=False,
                   num_devices=NCORES)

    xq_d = nc.dram_tensor("xq", [C, NQ], F32R, kind="ExternalInput")
    aux_d = nc.dram_tensor("aux", [C, N], F32R, kind="ExternalInput")
    xb_d = nc.dram_tensor("xb", [C, N], BF16, kind="ExternalInput")
    wqT_d = nc.dram_tensor("wqT", [C, CH], F32R, kind="ExternalInput")
    wkT_d = nc.dram_tensor("wkT", [C, CH], F32R, kind="ExternalInput")
    wvT_d = nc.dram_tensor("wvT", [C, CH], BF16, kind="ExternalInput")
    wzT_d = nc.dram_tensor("wzT", [CH, C], F32R, kind="ExternalInput")
    bq_d = nc.dram_tensor("bq", [CH, 1], F32, kind="ExternalInput")
    bk_d = nc.dram_tensor("bk", [CH, 1], F32, kind="ExternalInput")
    bv_d = nc.dram_tensor("bv", [CH, 1], F32, kind="ExternalInput")
    bz_d = nc.dram_tensor("bz", [C, 1], F32, kind="ExternalInput")
    z_d = nc.dram_tensor("z", [C, NQ], F32, kind="ExternalOutput")

    with tile.TileContext(nc) as tc:
        with (
            tc.tile_pool(name="const", bufs=1) as constp,
            tc.tile_pool(name="persist", bufs=1) as persist,
            tc.tile_pool(name="dram", bufs=10, space="DRAM") as dramp,
            tc.tile_pool(name="E", bufs=21) as Ep,
            tc.tile_pool(name="rp", bufs=4) as rp,
            tc.tile_pool(name="auxp", bufs=5) as auxp,
            tc.tile_pool(name="xbp", bufs=5) as xbp,
            tc.tile_pool(name="vch", bufs=4) as vchp,
            tc.tile_pool(name="zt", bufs=4) as ztp,
        ):
            # ---- exp table primer: load the ACT exp tables at t=0 so the
            # ~2.7us table load overlaps the input DMAs.
            ebias = constp.tile([128, 1], F32, tag="ebias", name="ebias")
            nc.vector.memset(ebias[:], EXP_BIAS)
            prim = constp.tile([128, 1], F32, tag="prim", name="prim")
            nc.vector.memset(prim[:], 0.0)
            primo = constp.tile([128, 1], F32, tag="primo", name="primo")
            nc.scalar.activation(primo[:], prim[:], AF.Exp, bias=ebias[:],
                                 scale=1.0)

            # ---- collective warm-up: absorbs first-collective setup +
            # core-start skew while the input DMAs proceed.
            warm_sb = constp.tile([1, 4], F32, tag="warm", name="warm_sb")
            nc.vector.memset(warm_sb[:], 1.0)
            warm_in = dramp.tile([1, 4], F32, tag="warmin", name="warmin")
            warm_out = dramp.tile([1, 4], F32, tag="warmout", name="warmout")
            nc.sync.dma_start(warm_in[:], warm_sb[:])
            nc.gpsimd.collective_compute(
                "AllReduce", ALU.add, replica_groups=GROUPS,
                ins=[warm_in.opt()], outs=[warm_out.opt()])

            # ---- constant tiles ----
            wqT = [constp.tile([128, CH], F32R, tag=f"wq{i}", name=f"wq{i}")
                   for i in range(2)]
            wkT = [constp.tile([128, CH], F32R, tag=f"wk{i}", name=f"wk{i}")
                   for i in range(2)]
            wvT = [constp.tile([128, CH], BF16, tag=f"wv{i}", name=f"wv{i}")
                   for i in range(2)]
            wzT_sb = constp.tile([128, C], F32R, tag="wz", name="wzT_sb")
            bq_sb = constp.tile([CH, 1], F32, tag="bq", name="bq_sb")
            bk_sb = constp.tile([CH, 1], F32, tag="bk", name="bk_sb")
            bv_sb = constp.tile([CH, 1], F32, tag="bv", name="bv_sb")
            bz_sb = [constp.tile([128, 1], F32, tag=f"bz{i}", name=f"bz{i}")
                     for i in range(2)]
            ident0 = constp.tile([128, 128], F32, tag="ident0", name="ident0")
            make_identity(nc, ident0[:])
            ident = constp.tile([128, 128], BF16, tag="ident", name="ident")
            nc.vector.tensor_copy(ident[:], ident0[:])

            # ---- persistent activations ----
            xq_sb = [persist.tile([128, NQ], F32R, tag=f"xq{i}",
                                  name=f"xq{i}") for i in range(2)]
            K_sb = persist.tile([128, N], F32R, tag="K", name="K_sb")
            Q_sb = persist.tile([128, NQ], F32R, tag="Q", name="Q_sb")
            Vt = [persist.tile([128, CH], BF16, tag=f"vt{g}", name=f"vt{g}")
                  for g in range(32)]
            y_sb = persist.tile([128, NQ], F32R, tag="y", name="y_sb")

            # ---- input DMAs, priority-ordered (sync queue) ----
            # Critical path to the first exp: wk, wq, aux cols 0:512, xq.
            for i in range(2):
                nc.sync.dma_start(wkT[i][:], wkT_d[i * 128:(i + 1) * 128, :])
            for i in range(2):
                nc.sync.dma_start(wqT[i][:], wqT_d[i * 128:(i + 1) * 128, :])
            nc.sync.dma_start(bk_sb[:], bk_d[:, :])
            nc.sync.dma_start(bq_sb[:], bq_d[:, :])
            aux_t = {}  # (tile_idx, ci) -> sbuf piece
            def load_aux(t):
                for i in range(2):
                    a = auxp.tile([128, QT], F32R, tag=f"a{i}",
                                  name=f"aux{t}_{i}")
                    nc.sync.dma_start(
                        a[:], aux_d[i * 128:(i + 1) * 128,
                                    t * QT:(t + 1) * QT])
                    aux_t[(t, i)] = a
            load_aux(0)
            # xq in 512-column pieces so each Q projection tile can start
            # as soon as its slice lands.
            for t in range(4):
                for i in range(2):
                    nc.sync.dma_start(
                        xq_sb[i][:, t * QT:(t + 1) * QT],
                        xq_d[i * 128:(i + 1) * 128, t * QT:(t + 1) * QT])
            # Secondary: remaining aux tiles, V-path inputs, z weights.
            for t in (1, 2, 3):
                load_aux(t)
            nc.sync.dma_start(bv_sb[:], bv_d[:, :])
            for i in range(2):
                nc.sync.dma_start(wvT[i][:], wvT_d[i * 128:(i + 1) * 128, :])
            xb_t = {}
            def load_xb(kc):
                for i in range(2):
                    xbt = xbp.tile([128, QT], BF16, tag=f"xb{i}",
                                   name=f"xb{kc}_{i}")
                    nc.sync.dma_start(
                        xbt[:], xb_d[i * 128:(i + 1) * 128,
                                     kc * QT:(kc + 1) * QT])
                    xb_t[(kc, i)] = xbt
            for kc in range(4):
                load_xb(kc)
            for t in (4, 5, 6, 7):
                load_aux(t)
            for kc in range(4, 8):
                load_xb(kc)
            nc.sync.dma_start(wzT_sb[:], wzT_d[:, :])
            for i in range(2):
                nc.sync.dma_start(bz_sb[i][:], bz_d[i * 128:(i + 1) * 128, :])

            # ---- head projections: K tile 0 + all of Q (own PSUM pool,
            # closed before the main slot pool claims all 8 banks).
            def proj_tile(ps, w01, src01, bias, dst, dsl):
                nc.tensor.matmul(ps[:], w01[0][:], src01[0][:],
                                 start=True, stop=False)
                nc.tensor.matmul(ps[:], w01[1][:], src01[1][:],
                                 start=False, stop=True)
                nc.vector.tensor_scalar_add(dst[:, dsl], ps[:], bias[:])

            with tc.tile_pool(name="hd_ps", bufs=2, space="PSUM") as hdps:
                ps = hdps.tile([128, QT], F32, tag="hd", name="hd_k0")
                proj_tile(ps, wkT, [aux_t[(0, 0)], aux_t[(0, 1)]], bk_sb,
                          K_sb, slice(0, QT))
                for t in range(4):
                    ps = hdps.tile([128, QT], F32, tag="hd", name=f"hd_q{t}")
                    sl = slice(t * QT, (t + 1) * QT)
                    proj_tile(ps, wqT,
                              [xq_sb[0][:, sl], xq_sb[1][:, sl]], bq_sb,
                              Q_sb, sl)

            # ---- main pipeline ----
            # One PSUM pool: two [128, 2048] slots (all 8 banks).  The
            # rotation carries S subchunks, V / late-K projections, the
            # per-chunk y accumulators, and finally the z projections.
            # Every slot's consumers are emitted immediately after its
            # allocation so the 2-buf rotation can never deadlock.
            slotp_cm = tc.tile_pool(name="slot", bufs=2, space="PSUM")
            slotp = slotp_cm.__enter__()

            def slot_alloc(name):
                return slotp.tile([128, 4 * QT], F32, tag="slot", name=name)

            # V projection block: one slot covers 2 key chunks -- for each,
            # a V matmul pair into one quarter, then four PE transposes of
            # the biased bf16 V into the adjacent quarter, evacuated to the
            # persistent Vt tiles.
            def emit_v_block(kcs, name):
                vslot = slot_alloc(name)
                for j, kc in enumerate(kcs):
                    qsl = slice(2 * j * QT, (2 * j + 1) * QT)
                    nc.tensor.matmul(vslot[:, qsl], wvT[0][:],
                                     xb_t[(kc, 0)][:], start=True, stop=False)
                    nc.tensor.matmul(vslot[:, qsl], wvT[1][:],
                                     xb_t[(kc, 1)][:], start=False, stop=True)
                for j, kc in enumerate(kcs):
                    qsl = slice(2 * j * QT, (2 * j + 1) * QT)
                    vch = vchp.tile([128, QT], BF16, tag="vch",
                                    name=f"vch{kc}")
                    nc.vector.tensor_scalar_add(vch[:], vslot[:, qsl],
                                                bv_sb[:])
                    tq = vslot[:, (2 * j + 1) * QT:(2 * j + 2) * QT]
                    tq16 = tq.bitcast(BF16)
                    for s in range(NSUB):
                        nc.tensor.transpose(tq16[:, s * 128:(s + 1) * 128],
                                            vch[:, s * 128:(s + 1) * 128],
                                            ident[:])
                    for s in range(NSUB):
                        g = kc * NSUB + s
                        nc.vector.tensor_copy(
                            Vt[g][:], tq16[:, s * 128:(s + 1) * 128])

            # Late-K projection block: one slot covers up to 4 K tiles.
            def emit_kl_block(ts, name):
                kslot = slot_alloc(name)
                for j, t in enumerate(ts):
                    qsl = slice(j * QT, (j + 1) * QT)
                    nc.tensor.matmul(kslot[:, qsl], wkT[0][:],
                                     aux_t[(t, 0)][:], start=True, stop=False)
                    nc.tensor.matmul(kslot[:, qsl], wkT[1][:],
                                     aux_t[(t, 1)][:], start=False, stop=True)
                for j, t in enumerate(ts):
                    qsl = slice(j * QT, (j + 1) * QT)
                    nc.vector.tensor_scalar_add(
                        K_sb[:, t * QT:(t + 1) * QT], kslot[:, qsl], bk_sb[:])

            E_c = {}        # chunk -> list of 4 E strips
            r_c = {}        # chunk -> r partial tile [128, 4]
            ar_out = {}     # AR name -> (rout dram tile, kcs)
            rred_c = {}     # chunk -> (reduced rowsum tile, col offset)
            y_c = {}        # chunk -> open y slot

            # Rowsum AllReduce, split into a trigger (staging DMAs on the
            # idle sync queue + gpsimd collective kick) and a deferred
            # readback, so collectives overlap in flight instead of
            # serializing the gpsimd queue on each one's completion.
            def ar_trigger(kcs, name):
                n = len(kcs)
                rin = dramp.tile([128, NSUB * n], F32, tag=f"rin{name}",
                                 name=f"rin{name}")
                rout = dramp.tile([128, NSUB * n], F32, tag=f"rout{name}",
                                  name=f"rout{name}")
                for j, c in enumerate(kcs):
                    nc.gpsimd.dma_start(rin[:, NSUB * j:NSUB * (j + 1)],
                                        r_c[c][:])
                nc.gpsimd.collective_compute(
                    "AllReduce", ALU.add, replica_groups=GROUPS,
                    ins=[rin.opt()], outs=[rout.opt()])
                ar_out[name] = (rout, kcs)

            def ar_read(name):
                rout, kcs = ar_out.pop(name)
                n = len(kcs)
                rr = rp.tile([128, NSUB * n], F32, tag=f"rred{name}",
                             name=f"rred{name}")
                nc.gpsimd.dma_start(rr[:], rout[:])
                for j, c in enumerate(kcs):
                    rred_c[c] = (rr, NSUB * j)

            def emit_scale(kc):
                rr, off = rred_c.pop(kc)
                rinv = rp.tile([128, NSUB], F32, tag="rinv",
                               name=f"rinv{kc}")
                nc.vector.reciprocal(rinv[:], rr[:, off:off + NSUB])
                for s in range(NSUB):
                    g = kc * NSUB + s
                    nc.vector.tensor_scalar_mul(Vt[g][:], Vt[g][:],
                                                rinv[:, s:s + 1])

            def emit_y_qt(kc, qt):
                yslot = y_c[kc]
                qsl = slice(qt * QT, (qt + 1) * QT)
                E = E_c[kc]
                for s in range(NSUB):
                    nc.tensor.matmul(yslot[:, qsl], Vt[kc * NSUB + s][:],
                                     E[s][:, qsl],
                                     start=(s == 0), stop=(s == 3))
                if kc == 0:
                    nc.vector.tensor_copy(y_sb[:, qsl], yslot[:, qsl])
                else:
                    nc.vector.tensor_add(y_sb[:, qsl],
                                         y_sb[:, qsl].bitcast(F32),
                                         yslot[:, qsl])

            def emit_y_first(kc):
                y_c[kc] = slot_alloc(f"y{kc}")
                emit_y_qt(kc, 0)
                emit_y_qt(kc, 1)

            def emit_y_second(kc):
                emit_y_qt(kc, 2)
                emit_y_qt(kc, 3)
                del y_c[kc]
                del E_c[kc]

            # ---- attention loop ----
            for kc in range(NCHUNK):
                if kc >= 5:
                    emit_y_second(kc - 5)
                E = [Ep.tile([128, NQ], BF16, tag="E", name=f"E{kc}_{s}")
                     for s in range(NSUB)]
                E_c[kc] = E
                rc = rp.tile([128, NSUB], F32, tag="rpart", name=f"rc{kc}")
                r_c[kc] = rc
                for sp in range(NSUB // 2):
                    # Two subchunks per step: the eight 512-wide S matmuls
                    # alternate stationary K tiles so LDWEIGHTS pipelines
                    # into the background weight buffer instead of
                    # serializing with the in-flight matmul.
                    s0, s1 = 2 * sp, 2 * sp + 1
                    ksl0 = slice((kc * NSUB + s0) * 128,
                                 (kc * NSUB + s0 + 1) * 128)
                    ksl1 = slice((kc * NSUB + s1) * 128,
                                 (kc * NSUB + s1 + 1) * 128)
                    slot0 = slot_alloc(f"s{kc}_{s0}")
                    slot1 = slot_alloc(f"s{kc}_{s1}")
                    for j in range(4):
                        qsl = slice(j * QT, (j + 1) * QT)
                        nc.tensor.matmul(slot0[:, qsl], K_sb[:, ksl0],
                                         Q_sb[:, qsl], start=True, stop=True)
                        nc.tensor.matmul(slot1[:, qsl], K_sb[:, ksl1],
                                         Q_sb[:, qsl], start=True, stop=True)
                    nc.scalar.activation(E[s0][:], slot0[:], AF.Exp,
                                         bias=ebias[:], scale=1.0,
                                         accum_out=rc[:, s0:s0 + 1])
                    nc.scalar.activation(E[s1][:], slot1[:], AF.Exp,
                                         bias=ebias[:], scale=1.0,
                                         accum_out=rc[:, s1:s1 + 1])
                    s = s1  # insert schedule below keys off s in {1, 3}
                    if kc >= 4 and s == 3:
                        emit_scale(kc - 4)
                    if kc == 0 and s == 1:
                        emit_v_block([0, 1], "vA")
                    if kc == 0 and s == 3:
                        emit_kl_block([1, 2, 3], "klA")
                    if kc == 1 and s == 1:
                        emit_v_block([2, 3], "vB")
                    if kc == 1 and s == 3:
                        emit_kl_block([4, 5], "klB")
                    if kc == 2 and s == 1:
                        emit_v_block([4, 5], "vC")
                    if kc == 3 and s == 1:
                        emit_kl_block([6, 7], "klC")
                    if kc == 3 and s == 3:
                        emit_v_block([6, 7], "vD")
                # chunk boundary: collectives and lagged y sessions
                if kc == 1:
                    ar_trigger([0, 1], "A")
                if kc == 3:
                    ar_trigger([2, 3], "B")
                    ar_read("A")
                if kc == 5:
                    ar_read("B")
                if kc == 6:
                    ar_trigger([4, 5, 6], "C")
                if kc == 7:
                    ar_trigger([7], "D")
                    ar_read("C")
                if kc >= 4:
                    emit_y_first(kc - 4)

            # ---- tail ----
            emit_y_second(3)
            emit_scale(4)
            emit_y_first(4)
            emit_y_second(4)
            emit_scale(5)
            emit_y_first(5)
            emit_y_second(5)
            emit_scale(6)
            emit_y_first(6)
            emit_y_second(6)
            ar_read("D")
            emit_scale(7)
            emit_y_first(7)
            emit_y_second(7)

            # z projection + residual, per 512-query tile
            for half in range(2):
                zslot = slot_alloc(f"z{half}")
                for qi in range(2):
                    qt = half * 2 + qi
                    qsl = slice(qt * QT, (qt + 1) * QT)
                    for co in range(2):
                        psl = slice((qi * 2 + co) * QT,
                                    (qi * 2 + co + 1) * QT)
                        nc.tensor.matmul(
                            zslot[:, psl],
                            wzT_sb[:, co * 128:(co + 1) * 128],
                            y_sb[:, qsl], start=True, stop=True)
                        zt = ztp.tile([128, QT], F32, tag="zt",
                                      name=f"zt{qt}_{co}")
                        nc.vector.scalar_tensor_tensor(
                            zt[:], zslot[:, psl], bz_sb[co][:],
                            xq_sb[co][:, qsl].bitcast(F32),
                            op0=ALU.add, op1=ALU.add)
                        nc.sync.dma_start(
                            z_d[co * 128:(co + 1) * 128, qsl], zt[:])

            slotp_cm.__exit__(None, None, None)

    nc.compile()
    return nc


def make_in_maps(inputs: dict) -> list:
    import ml_dtypes
    x = np.ascontiguousarray(np.asarray(inputs["x"], np.float32)
                             .reshape(B, C, N))
    aux = np.ascontiguousarray(np.asarray(inputs["aux"], np.float32)
                               .reshape(B, C, N))
    xb = x.astype(ml_dtypes.bfloat16)
    wqT = np.ascontiguousarray(np.asarray(inputs["wq_w"], np.float32).T)
    wkT = np.ascontiguousarray(np.asarray(inputs["wk_w"], np.float32).T)
    wvT = np.ascontiguousarray(
        np.asarray(inputs["wv_w"], np.float32).T.astype(ml_dtypes.bfloat16))
    wzT = np.ascontiguousarray(np.asarray(inputs["wz_w"], np.float32).T)
    bq = np.asarray(inputs["wq_b"], np.float32).reshape(CH, 1)
    bk = np.asarray(inputs["wk_b"], np.float32).reshape(CH, 1)
    bv = np.asarray(inputs["wv_b"], np.float32).reshape(CH, 1)
    bz = np.asarray(inputs["wz_b"], np.float32).reshape(C, 1)
    in_maps = []
    for c in range(NCORES):
        b, h = c // 2, c % 2
        in_maps.append({
            "xq": np.ascontiguousarray(x[b][:, h * NQ:(h + 1) * NQ]),
            "aux": aux[b],
            "xb": xb[b],
            "wqT": wqT, "wkT": wkT, "wvT": wvT, "wzT": wzT,
            "bq": bq, "bk": bk, "bv": bv, "bz": bz,
        })
    return in_maps


class Runner:
    """Compile once, then run the SPMD kernel any number of times.

    Mirrors bass2jax.run_bass_via_pjrt's multi-core branch but keeps the
    jitted executable so repeated calls don't re-trace/re-compile.
    """

    def __init__(self, nc=None):
        import jax
        from jax.experimental.shard_map import shard_map
        from jax.sharding import Mesh, PartitionSpec

        self.nc = nc if nc is not None else build_program()
        bass2jax.install_neuronx_cc_hook()
        nc = self.nc
        assert nc.dbg_addr is None
        partition_name = (nc.partition_id_tensor.name
                          if nc.partition_id_tensor else None)

        in_names, out_names, out_avals, zero_outs = [], [], [], []
        for alloc in nc.m.functions[0].allocations:
            if not isinstance(alloc, mybir.MemoryLocationSet):
                continue
            name = alloc.memorylocations[0].name
            if alloc.kind == "ExternalInput":
                if name != partition_name:
                    in_names.append(name)
            elif alloc.kind == "ExternalOutput":
                out_names.append(name)
                shape = tuple(alloc.tensor_shape)
                dtype = mybir.dt.np(alloc.dtype)
                out_avals.append(jax.core.ShapedArray(shape, dtype))
                zero_outs.append(np.zeros(shape, dtype))
        self.in_names = list(in_names)
        self.out_names = out_names
        self.out_avals = out_avals
        n_params = len(in_names)
        n_outs = len(out_avals)
        all_names = in_names + out_names
        if partition_name is not None:
            all_names = all_names + [partition_name]

        def _body(*args):
            operands = list(args)
            if partition_name is not None:
                operands.append(bass2jax.partition_id_tensor())
            outs = bass2jax._bass_exec_p.bind(
                *operands,
                out_avals=tuple(out_avals),
                in_names=tuple(all_names),
                out_names=tuple(out_names),
                lowering_input_output_aliases=(),
                sim_require_finite=True,
                sim_require_nnan=True,
                nc=nc,
            )
            return tuple(outs)

        devices = jax.devices()[:NCORES]
        mesh = Mesh(np.asarray(devices), ("core",))
        from jax.sharding import NamedSharding
        self._sharding = NamedSharding(mesh, PartitionSpec("core"))
        in_specs = (PartitionSpec("core"),) * (n_params + n_outs)
        out_specs = (PartitionSpec("core"),) * n_outs
        self._sharded = jax.jit(
            shard_map(_body, mesh=mesh, in_specs=in_specs,
                      out_specs=out_specs, check_rep=False),
            donate_argnums=tuple(range(n_params, n_params + n_outs)),
            keep_unused=True,
        )
        self._zero_outs = zero_outs

    def device_inputs(self, in_maps):
        """Transfer the concatenated per-core inputs to the devices once."""
        import jax

        concat_in = [
            np.concatenate([np.asarray(in_maps[c][name])
                            for c in range(NCORES)], axis=0)
            for name in self.in_names
        ]
        return [jax.device_put(a, self._sharding) for a in concat_in]

    def run_device(self, dev_in):
        """Execute with device-resident inputs; returns device arrays."""
        concat_zeros = [
            np.zeros((NCORES * z.shape[0], *z.shape[1:]), z.dtype)
            for z in self._zero_outs
        ]
        return self._sharded(*dev_in, *concat_zeros)

    def run(self, in_maps):
        out_arrs = self.run_device(self.device_inputs(in_maps))
        return [
            {
                name: np.asarray(out_arrs[i]).reshape(
                    NCORES, *self.out_avals[i].shape)[c]
                for i, name in enumerate(self.out_names)
            }
            for c in range(NCORES)
        ]


_RUNNER = None


def get_runner() -> Runner:
    global _RUNNER
    if _RUNNER is None:
        _RUNNER = Runner()
    return _RUNNER


def assemble(results) -> np.ndarray:
    out = np.empty((B, C, N), np.float32)
    for c in range(NCORES):
        b, h = c // 2, c % 2
        out[b][:, h * NQ:(h + 1) * NQ] = results[c]["z"]
    return out.reshape(B, C, 64, 64)


def kernel(**inputs) -> np.ndarray:
    runner = get_runner()
    results = runner.run(make_in_maps(inputs))
    return assemble(results)
